# revision 5
# baseline (speedup 1.0000x reference)
"""Trainium2 Bass kernel for nn_Compute_all_u (embedding gather + batched affine dot).

Computes, for each voxel v:
    u[v, :] = coeffs[e_v, 0, :] + x_v*coeffs[e_v, 1, :] + y_v*coeffs[e_v, 2, :] + z_v*coeffs[e_v, 3, :]
where e_v = voxels_elements[v], (x,y,z) = all_voxels_centroids[v].

Sharding: data-parallel over the voxel axis across 8 NeuronCores.

Gather mechanism: InstDMAGatherAnt (`nc.gpsimd.dma_gather`) — one SWDGE
instruction gathers thousands of table rows, amortizing the ~1us
fixed per-instruction SWDGE cost that made the per-128-row indirect-DMA
approach descriptor-generation-bound. Its two constraints are bridged
host-side:

  * int16 indices -> the 500k-row table is viewed as 16 slabs of 32768
    rows; voxels are host-partitioned by slab and carry slab-local
    indices (0..32767).
  * 256B-multiple element size -> table rows are padded 12 -> 64 floats
    (48B -> 256B) in a host-built padded table.

Voxels are reordered on the host (grouped by slab, padded per slab to a
tile multiple) and the device output is un-permuted on the host.

Device layout per gather tile of T voxels (tile element i):
  index  at idx[t, i%16 (replicated x8 across partitions), i//16]  (int16)
  output at out[t, i%128, i//128, :]
so padded voxel position v = tile_base + g*128 + p maps to (p, g).
"""

import numpy as np

from concourse import bacc, bass, tile, mybir
from concourse.bass_utils import run_bass_kernel_spmd

N_VOXELS = 8_000_000
N_ELEM = 500_000
N_CORES = 8
P = 128

CHUNK = 32768                          # table rows per slab (int16 index range)
NCHUNK = (N_ELEM + CHUNK - 1) // CHUNK  # 16
TBL_PAD_ROWS = NCHUNK * CHUNK           # 524288
ELEM = 64                               # padded row: 64 f32 = 256 B
T = 4096                                # voxels per dma_gather instruction
G = T // P                              # gather-output columns per tile
NPC = N_VOXELS // N_CORES               # 1_000_000 voxels per core


def build_nc(tiles_chunk: tuple, bufs: int = 4) -> bass.Bass:
    # Bacc (not raw Bass): its compile pass splits multi-sem waits into
    # event semaphores — the TRN2 ISA allows at most one wait per
    # instruction and walrus codegen rejects Tile's raw output otherwise.
    nc = bacc.Bacc("TRN2")
    f32 = mybir.dt.float32
    i16 = mybir.dt.int16
    NT = sum(tiles_chunk)

    idx_in = nc.declare_dram_parameter("idx", [NT, P, T // 16], i16, isOutput=False)
    cent_in = nc.declare_dram_parameter("cent", [NT, P, 3 * G], f32, isOutput=False)
    table = nc.declare_dram_parameter("table", [NCHUNK, CHUNK, ELEM], f32, isOutput=False)
    out = nc.declare_dram_parameter("out", [NT, P, 3 * G], f32, isOutput=True)

    mul = mybir.AluOpType.mult
    add = mybir.AluOpType.add

    with tile.TileContext(nc) as tc:
        with (
            tc.tile_pool(name="io", bufs=bufs) as io_pool,
            tc.tile_pool(name="tmp", bufs=2) as tmp_pool,
        ):
            t = 0
            for c, ntc in enumerate(tiles_chunk):
                for _ in range(ntc):
                    idx_t = io_pool.tile([P, T // 16], i16, tag="idx")
                    nc.sync.dma_start(out=idx_t[:], in_=idx_in[t])

                    cent_t = io_pool.tile([P, 3 * G], f32, tag="cent")
                    nc.sync.dma_start(out=cent_t[:], in_=cent_in[t])

                    g = io_pool.tile([P, ELEM * G], f32, tag="g")
                    nc.gpsimd.dma_gather(
                        out_ap=g[:].rearrange("p (g e) -> p g e", e=ELEM),
                        in_ap=table[c],
                        idxs_ap=idx_t[:],
                        num_idxs=T,
                        num_idxs_reg=T,
                        elem_size=ELEM,
                        # default single_packet=True coalesces the whole
                        # stream into one SDMA packet; >64-desc packets
                        # (num_idxs > 1024) hard-fault the DMA engine.
                        single_packet=False,
                    )

                    # gathered row layout per voxel: [d=0..3][j=0..2] then pad
                    gr = g[:].rearrange("p (g e) -> p g e", e=ELEM)
                    cr = cent_t[:].rearrange("p (g j) -> p g j", j=3)

                    u = io_pool.tile([P, 3 * G], f32, tag="u")
                    ur = u[:].rearrange("p (g j) -> p g j", j=3)
                    tmp = tmp_pool.tile([P, 3 * G], f32, tag="t")
                    tr = tmp[:].rearrange("p (g j) -> p g j", j=3)

                    x_b = cr[:, :, 0:1].to_broadcast([P, G, 3])
                    y_b = cr[:, :, 1:2].to_broadcast([P, G, 3])
                    z_b = cr[:, :, 2:3].to_broadcast([P, G, 3])

                    nc.vector.tensor_tensor(out=tr, in0=x_b, in1=gr[:, :, 3:6], op=mul)
                    nc.vector.tensor_tensor(out=ur, in0=gr[:, :, 0:3], in1=tr, op=add)
                    nc.vector.tensor_tensor(out=tr, in0=y_b, in1=gr[:, :, 6:9], op=mul)
                    nc.vector.tensor_tensor(out=ur, in0=ur, in1=tr, op=add)
                    nc.vector.tensor_tensor(out=tr, in0=z_b, in1=gr[:, :, 9:12], op=mul)
                    nc.vector.tensor_tensor(out=ur, in0=ur, in1=tr, op=add)

                    nc.sync.dma_start(out=out[t], in_=u[:])
                    t += 1
    nc.finalize()
    return nc


_NC_CACHE: dict = {}


def _get_nc(tiles_chunk: tuple):
    if tiles_chunk not in _NC_CACHE:
        _NC_CACHE[tiles_chunk] = build_nc(tiles_chunk)
    return _NC_CACHE[tiles_chunk]


def _prepare(all_coeffs, all_voxels_centroids, voxels_elements):
    table = np.zeros((TBL_PAD_ROWS, ELEM), np.float32)
    table[:N_ELEM, :12] = np.asarray(all_coeffs, np.float32).reshape(N_ELEM, 12)
    table_dev = table.reshape(NCHUNK, CHUNK, ELEM)

    idx_all = np.asarray(voxels_elements).astype(np.int32).reshape(N_CORES, NPC)
    cent_all = np.asarray(all_voxels_centroids, np.float32).reshape(N_CORES, NPC, 3)

    ch = idx_all // CHUNK                   # slab id per voxel, [8, NPC]
    orders = [np.argsort(ch[c], kind="stable") for c in range(N_CORES)]
    counts = np.stack(
        [np.bincount(ch[c], minlength=NCHUNK) for c in range(N_CORES)]
    )                                       # [8, 16]
    caps = (counts.max(axis=0) + T - 1) // T * T    # per-slab capacity, T-multiple
    tiles_chunk = tuple(int(x) for x in caps // T)
    NT = int(caps.sum()) // T
    offs = np.concatenate([[0], np.cumsum(caps)]).astype(np.int64)

    in_maps = []
    metas = []
    for c in range(N_CORES):
        idx_pad = np.zeros(NT * T, np.int16)
        cent_pad = np.zeros((NT * T, 3), np.float32)
        order = orders[c]
        pad_pos = np.empty(NPC, np.int64)   # padded position of sorted voxel
        start = 0
        for k in range(NCHUNK):
            n = int(counts[c, k])
            o = order[start:start + n]
            idx_pad[offs[k]:offs[k] + n] = (idx_all[c][o] - k * CHUNK).astype(np.int16)
            cent_pad[offs[k]:offs[k] + n] = cent_all[c][o]
            pad_pos[start:start + n] = np.arange(offs[k], offs[k] + n)
            start += n
        idx_dev = np.ascontiguousarray(
            np.tile(idx_pad.reshape(NT, T // 16, 16).transpose(0, 2, 1), (1, 8, 1))
        )
        cent_dev = np.ascontiguousarray(
            cent_pad.reshape(NT, G, P, 3).transpose(0, 2, 1, 3)
        ).reshape(NT, P, 3 * G)
        in_maps.append({"idx": idx_dev, "cent": cent_dev, "table": table_dev})
        metas.append((order, pad_pos))
    return tiles_chunk, in_maps, metas


def kernel(all_coeffs, all_voxels_centroids, voxels_elements, _trace=False, **run_kwargs):
    tiles_chunk, in_maps, metas = _prepare(
        all_coeffs, all_voxels_centroids, voxels_elements
    )
    nc = _get_nc(tiles_chunk)
    res = run_bass_kernel_spmd(
        nc, in_maps, core_ids=list(range(N_CORES)), trace=_trace, **run_kwargs
    )
    NT = sum(tiles_chunk)
    full = np.empty((N_VOXELS, 3), np.float32)
    for c in range(N_CORES):
        o = (
            res.results[c]["out"]
            .reshape(NT, P, G, 3)
            .transpose(0, 2, 1, 3)
            .reshape(NT * T, 3)
        )
        order, pad_pos = metas[c]
        full[c * NPC + order] = o[pad_pos]
    if _trace:
        return full, res
    return full


# revision 7
# speedup vs baseline: 13.9801x; 13.9801x over previous
"""Trainium2 Bass kernel for nn_Compute_all_u (embedding gather + batched affine dot).

Computes, for each voxel v:
    u[v, :] = C[e_v,0,:] + x_v*C[e_v,1,:] + y_v*C[e_v,2,:] + z_v*C[e_v,3,:]
where e_v = voxels_elements[v], (x,y,z) = all_voxels_centroids[v].

Sharding: data-parallel over the voxel axis across 8 NeuronCores.

The expensive part of this problem is the irregular gather. A naive
per-row indirect DMA is descriptor-generation-bound on the GpSimd Q7
cores (~8ns/descriptor, serialized on the engine), so the design
minimizes DESCRIPTOR COUNT:

  * The coeff table is cast to bf16; one row = 24B, and lcm(24B, 256B)
    = 768B = exactly 32 rows. dma_gather elements are 768B 32-row
    GROUPS (elem_size=384 bf16) — descriptor stride/size constraints
    (multiples of 256B) are satisfied with zero padding, and group ids
    (0..15624) fit the gather's int16 index requirement outright: no
    table chunking.
  * Host-side, voxels are sorted by table row and packed into group
    INSTANCES: each gathered element carries DUP x 32 voxel slots,
    slot (dupk, j) serving one voxel whose row is group_base+j. A
    group needs ceil(max_row_multiplicity/DUP) instances. With random
    indices this is ~1.008 instances/group: ~15.7k descriptors/core
    for 1M voxels.
  * Extraction on-device is FREE: slot (dupk, j) reads the gathered
    element at static float offset 12*j, expressed as a strided
    (broadcast-over-dupk) access pattern in the DVE ops. No selects.

The device then does 6 tensor_tensor ops per tile (the affine combine)
and streams centroids in / u out, laid out host-side to match the
gather's (partition = element%128) layout. Host un-permutes the output.

Whole pipeline is bf16 (measured rel err vs f32 reference: 7.8e-3,
tolerance 2e-2).
"""

import numpy as np
import ml_dtypes

from concourse import bacc, bass, tile, mybir
from concourse.bass_utils import run_bass_kernel_spmd

BF16 = ml_dtypes.bfloat16

N_VOXELS = 8_000_000
N_ELEM = 500_000
N_CORES = 8
P = 128

NROW = 32                 # table rows per gathered element (32*24B = 768B)
DUP = 8                   # voxel slots per row per element instance
ELEM = NROW * 12          # 384 bf16 values per element
NGRP = N_ELEM // NROW     # 15625 groups
E = 512                   # elements per dma_gather instruction
COLS = E // P             # gather-output columns per tile
SLOTS = COLS * DUP * NROW  # slots per partition per tile (x3 floats)
NPC = N_VOXELS // N_CORES


def build_nc(nt: int, bufs: int = 4) -> bass.Bass:
    # Bacc (not raw Bass): its compile pass splits multi-sem waits into
    # event semaphores — the TRN2 ISA allows at most one wait per
    # instruction and walrus codegen rejects Tile's raw output otherwise.
    nc = bacc.Bacc("TRN2")
    bf = mybir.dt.bfloat16
    i16 = mybir.dt.int16

    idx_in = nc.declare_dram_parameter("idx", [nt, P, E // 16], i16, isOutput=False)
    cent_in = nc.declare_dram_parameter("cent", [nt, P, SLOTS * 3], bf, isOutput=False)
    table = nc.declare_dram_parameter("table", [NGRP, ELEM], bf, isOutput=False)
    out = nc.declare_dram_parameter("out", [nt, P, SLOTS * 3], bf, isOutput=True)

    mul = mybir.AluOpType.mult
    add = mybir.AluOpType.add
    B5 = [P, COLS, DUP, NROW, 3]

    with tile.TileContext(nc) as tc:
        with (
            tc.tile_pool(name="io", bufs=bufs) as io_pool,
            tc.tile_pool(name="tmp", bufs=2) as tmp_pool,
        ):
            for t in range(nt):
                idx_t = io_pool.tile([P, E // 16], i16, tag="idx")
                nc.sync.dma_start(out=idx_t[:], in_=idx_in[t])

                cent_t = io_pool.tile([P, SLOTS * 3], bf, tag="cent")
                nc.sync.dma_start(out=cent_t[:], in_=cent_in[t])

                g = io_pool.tile([P, COLS * ELEM], bf, tag="g")
                nc.gpsimd.dma_gather(
                    out_ap=g[:].rearrange("p (e v) -> p e v", v=ELEM),
                    in_ap=table[:],
                    idxs_ap=idx_t[:],
                    num_idxs=E,
                    num_idxs_reg=E,
                    elem_size=ELEM,
                    # default single_packet=True coalesces the stream into
                    # one SDMA packet; >64-desc packets fault the engine.
                    single_packet=False,
                )

                gr = g[:].rearrange("p (e r c) -> p e r c", r=NROW, c=12)
                cr = cent_t[:].rearrange(
                    "p (e d r c) -> p e d r c", d=DUP, r=NROW, c=3
                )

                u = io_pool.tile([P, SLOTS * 3], bf, tag="u")
                ur = u[:].rearrange("p (e d r c) -> p e d r c", d=DUP, r=NROW, c=3)
                tmp = tmp_pool.tile([P, SLOTS * 3], bf, tag="t")
                tr = tmp[:].rearrange("p (e d r c) -> p e d r c", d=DUP, r=NROW, c=3)

                # DVE APs are limited to TENSOR3D (partition + 3 free dims),
                # so loop over dupk with 4-dim slices.
                B4 = [P, COLS, NROW, 3]
                c0 = gr[:, :, :, 0:3]
                c1 = gr[:, :, :, 3:6]
                c2 = gr[:, :, :, 6:9]
                c3 = gr[:, :, :, 9:12]
                for d in range(DUP):
                    ud = ur[:, :, d, :, :]
                    td = tr[:, :, d, :, :]
                    x_b = cr[:, :, d, :, 0:1].to_broadcast(B4)
                    y_b = cr[:, :, d, :, 1:2].to_broadcast(B4)
                    z_b = cr[:, :, d, :, 2:3].to_broadcast(B4)
                    nc.vector.tensor_tensor(out=td, in0=x_b, in1=c1, op=mul)
                    nc.vector.tensor_tensor(out=ud, in0=c0, in1=td, op=add)
                    nc.vector.tensor_tensor(out=td, in0=y_b, in1=c2, op=mul)
                    nc.vector.tensor_tensor(out=ud, in0=ud, in1=td, op=add)
                    nc.vector.tensor_tensor(out=td, in0=z_b, in1=c3, op=mul)
                    nc.vector.tensor_tensor(out=ud, in0=ud, in1=td, op=add)

                nc.sync.dma_start(out=out[t], in_=u[:])
    nc.finalize()
    return nc


_NC_CACHE: dict = {}


def _get_nc(nt: int):
    if nt not in _NC_CACHE:
        _NC_CACHE[nt] = build_nc(nt)
    return _NC_CACHE[nt]


def _prepare(all_coeffs, all_voxels_centroids, voxels_elements):
    table = (
        np.asarray(all_coeffs, np.float32)
        .reshape(N_ELEM, 12)
        .astype(BF16)
        .reshape(NGRP, ELEM)
    )

    idx_all = np.asarray(voxels_elements).astype(np.int32).reshape(N_CORES, NPC)
    cent_all = np.asarray(all_voxels_centroids, np.float32).reshape(N_CORES, NPC, 3)

    percore = []
    for c in range(N_CORES):
        r = idx_all[c]
        order = np.argsort(r, kind="stable")
        rs = r[order]
        rank = np.arange(NPC) - np.searchsorted(rs, rs)  # rank within row
        cnt = np.bincount(rs, minlength=N_ELEM).reshape(NGRP, NROW)
        ninst = -(-cnt.max(axis=1) // DUP)               # instances per group
        percore.append((order, rs, rank, ninst))

    n_elems = [int(pc[3].sum()) for pc in percore]
    NT = -(-max(n_elems) // E)
    NE_PAD = NT * E

    in_maps = []
    metas = []
    for c in range(N_CORES):
        order, rs, rank, ninst = percore[c]
        elem_base = np.concatenate([[0], np.cumsum(ninst)]).astype(np.int64)
        e_id = elem_base[rs >> 5] + rank // DUP
        dupk = rank % DUP
        j = rs & 31

        idx_elem = np.zeros(NE_PAD, np.int16)
        idx_elem[: n_elems[c]] = np.repeat(
            np.arange(NGRP, dtype=np.int16), ninst
        )

        t = e_id // E
        i = e_id % E
        slot = (((t * P + i % P) * COLS + i // P) * DUP + dupk) * NROW + j

        cent_flat = np.zeros((NT * P * SLOTS, 3), BF16)
        cent_flat[slot] = cent_all[c][order].astype(BF16)

        idx_dev = np.ascontiguousarray(
            np.tile(idx_elem.reshape(NT, E // 16, 16).transpose(0, 2, 1), (1, 8, 1))
        )
        in_maps.append(
            {
                "idx": idx_dev,
                "cent": cent_flat.reshape(NT, P, SLOTS * 3),
                "table": table,
            }
        )
        metas.append((order, slot))
    return NT, in_maps, metas


def kernel(all_coeffs, all_voxels_centroids, voxels_elements, _trace=False, **run_kwargs):
    NT, in_maps, metas = _prepare(
        all_coeffs, all_voxels_centroids, voxels_elements
    )
    nc = _get_nc(NT)
    res = run_bass_kernel_spmd(
        nc, in_maps, core_ids=list(range(N_CORES)), trace=_trace, **run_kwargs
    )
    full = np.empty((N_VOXELS, 3), np.float32)
    for c in range(N_CORES):
        o = res.results[c]["out"].reshape(NT * P * SLOTS, 3).astype(np.float32)
        order, slot = metas[c]
        full[c * NPC + order] = o[slot]
    if _trace:
        return full, res
    return full
